# revision 72
# baseline (speedup 1.0000x reference)
"""Channel-wise Linear on 8 TRN2 NeuronCores.

y[b, c, :] = x[b, c, :] @ W[c].T + b[c]   (B=64, C=128, F=1024, fp32 ref)

Sharding: channels split across 8 cores (16 each, expert-style), no
cross-core communication. Host pre-packs per-channel operands into
device-friendly layouts (contraction dim on partitions), in bf16:
  wh[c] = W[c].T tiles    [128, KT*F]      (moving matmul operand)
  xs    = x.T tiles       [128, CPC*KT*B]  (stationary operand)
  bs[c] = raw bias        [1, F]
Device, per channel: bias is seeded into PSUM via a K=1 matmul
(ones x bias-row broadcasts it over the batch partitions), then 8
K-tiles of xT.T @ WT accumulate on top in bf16 at full PE rate with
fp32 PSUM accumulation. PSUM is copied to SBUF on the vector engine,
four channels packed per [128, 2F] tile (pairs in rows x pairs in
columns) so output DMAs use 4 KB/partition descriptors; the host
gathers/transposes shards and upcasts to fp32.

Perf notes (measured on 8-core axon TRN2): the kernel is HBM-bound —
~36 MB/core streams at ~370 GB/s effective. fp32 matmul runs at 4
cycles/column on the PE (2 half-speed passes), so fp32 compute would
be slower than the bf16 DMA stream; bf16 keeps rel err ~2.6e-3.
A ~100-matmul warm-up chain runs during the initial W DMA so the PE
HAM clock-gate is at 8/8 before real work. W is fetched as two
[128, 4096] chunks per channel: one big dma_start fans out over only
half the HW queues, and finer chunks lower descriptor efficiency.
"""

import numpy as np
import ml_dtypes

import concourse.bass as bass
import concourse.bacc as bacc
import concourse.mybir as mybir
from concourse import tile
from concourse import bass_utils

B, C, F = 64, 128, 1024
NCORES = 8
CPC = C // NCORES          # channels per core
KT = F // 128              # contraction tiles per channel
F32 = mybir.dt.float32
F32R = mybir.dt.float32r

COMPUTE = "bf16"           # "bf16" | "f32" | "f32r"

_CACHE = {}


def _np_in_dtype():
    return ml_dtypes.bfloat16 if COMPUTE == "bf16" else np.float32


def _build():
    if "nc" in _CACHE:
        return _CACHE["nc"]
    wdt = mybir.dt.bfloat16 if COMPUTE == "bf16" else F32
    mm_cast = (lambda ap: ap.bitcast(F32R)) if COMPUTE == "f32r" else (lambda ap: ap)

    nc = bacc.Bacc(
        "TRN2",
        target_bir_lowering=False,
        debug=False,
        enable_asserts=True,
        num_devices=NCORES,
    )
    wh = nc.dram_tensor("wh", [CPC, 128, KT * F], wdt, kind="ExternalInput").ap()
    xs = nc.dram_tensor("xs", [128, CPC * KT * B], wdt, kind="ExternalInput").ap()
    bs = nc.dram_tensor("bs", [CPC, 1, F], wdt, kind="ExternalInput").ap()
    yc = nc.dram_tensor("yc", [CPC // 4, 128, 2 * F], wdt, kind="ExternalOutput").ap()

    with tile.TileContext(nc) as tc:
        with (
            tc.tile_pool(name="w", bufs=8) as wpool,
            tc.tile_pool(name="x", bufs=1) as xpool,
            tc.tile_pool(name="bi", bufs=3) as bpool,
            tc.tile_pool(name="one", bufs=1) as onepool,
            tc.tile_pool(name="o", bufs=3) as opool,
            tc.tile_pool(name="ps", bufs=8, space=bass.MemorySpace.PSUM) as pspool,
        ):
            ones = onepool.tile([1, B], wdt)
            nc.gpsimd.memset(ones[:], 1.0)

            # PE warm-up: ~100 tiny back-to-back matmuls run during the
            # initial W DMA wait so HAM unthrottles before real work.
            wu = pspool.tile([1, B], F32, tag="ps")
            for _ in range(100):
                nc.tensor.matmul(wu[:], ones[:, 0:1], ones[:], start=True, stop=True)

            x_all = xpool.tile([128, CPC * KT * B], wdt)
            nc.sync.dma_start(x_all[:], xs[:])

            o_t = None
            for c in range(CPC):
                x_t = x_all[:, c * KT * B:(c + 1) * KT * B]
                b_t = bpool.tile([1, F], wdt)
                nc.sync.dma_start(b_t[:], bs[c])
                w_t = wpool.tile([128, KT * F], wdt)
                half = KT * F // 2
                for j in range(2):
                    nc.sync.dma_start(
                        w_t[:, j * half:(j + 1) * half],
                        wh[c][:, j * half:(j + 1) * half],
                    )

                ps0 = pspool.tile([B, 512], F32, tag="ps")
                ps1 = pspool.tile([B, 512], F32, tag="ps")
                # bias seed: ps = ones.T @ bias_row (K=1)
                nc.tensor.matmul(
                    ps0[:], ones[:], b_t[:, 0:512],
                    start=True, stop=False, skip_group_check=True,
                )
                nc.tensor.matmul(
                    ps1[:], ones[:], b_t[:, 512:F],
                    start=True, stop=False, skip_group_check=True,
                )
                for kt in range(KT):
                    lhsT = mm_cast(x_t[:, kt * B:(kt + 1) * B])
                    wk = w_t[:, kt * F:(kt + 1) * F]
                    nc.tensor.matmul(
                        ps0[:], lhsT, mm_cast(wk[:, 0:512]),
                        start=False, stop=(kt == KT - 1), skip_group_check=True,
                    )
                    nc.tensor.matmul(
                        ps1[:], lhsT, mm_cast(wk[:, 512:F]),
                        start=False, stop=(kt == KT - 1), skip_group_check=True,
                    )

                # pack 4 channels per [128, 2F] out tile: rows split a pair,
                # columns split two pairs -> 4 KB/partition out descriptors
                if c % 4 == 0:
                    o_t = opool.tile([128, 2 * F], wdt)
                rows = slice(0, B) if c % 2 == 0 else slice(B, 2 * B)
                goff = (c % 4 // 2) * F
                nc.vector.tensor_copy(o_t[rows, goff:goff + 512], ps0[:])
                nc.vector.tensor_copy(o_t[rows, goff + 512:goff + F], ps1[:])
                if c % 4 == 3:
                    nc.sync.dma_start(yc[c // 4], o_t[:])

    nc.compile()
    _CACHE["nc"] = nc
    return nc


def shard_inputs(x, W, b):
    ndt = _np_in_dtype()
    in_maps = []
    for core in range(NCORES):
        cs, ce = core * CPC, (core + 1) * CPC
        # wh[c, p, kt*F + g] = W[c][g][kt*128 + p]
        wt = W[cs:ce].astype(ndt).transpose(0, 2, 1)          # [CPC, f, g]
        wh = np.ascontiguousarray(
            wt.reshape(CPC, KT, 128, F).transpose(0, 2, 1, 3)
        ).reshape(CPC, 128, KT * F)
        xt = x[:, cs:ce, :].astype(ndt).transpose(1, 2, 0)    # [CPC, f, b]
        xs = np.ascontiguousarray(
            xt.reshape(CPC, KT, 128, B).transpose(2, 0, 1, 3)
        ).reshape(128, CPC * KT * B)
        bs = np.ascontiguousarray(b[cs:ce].reshape(CPC, 1, F).astype(ndt))
        in_maps.append({"wh": wh, "xs": xs, "bs": bs})
    return in_maps


def gather_output(results):
    yc = np.stack([results[core]["yc"] for core in range(NCORES)])
    # [8, CPC//4, 128, 2F]: rows (i) split a channel pair, cols (j) split
    # two pairs; channel = quad*4 + 2*j + i
    y = yc.reshape(NCORES, CPC // 4, 2, B, 2, F)       # [core, q, i, b, j, g]
    y = y.transpose(0, 1, 4, 2, 3, 5).reshape(C, B, F)  # [core, q, j, i, b, g]
    return np.ascontiguousarray(y.transpose(1, 0, 2).astype(np.float32))


def kernel(x, W, b):
    x = np.asarray(x)
    W = np.asarray(W)
    b = np.asarray(b)
    nc = _build()
    in_maps = shard_inputs(x, W, b)
    res = bass_utils.run_bass_kernel_spmd(nc, in_maps, core_ids=list(range(NCORES)))
    return gather_output(res.results)
